# revision 1
# baseline (speedup 1.0000x reference)
"""Bass/Tile kernel for nn_CloudCrop (cylinder-query + gather + SharedMLP + max-pool).

Sharding: 8 cores = 4 batches x 2 query-halves. Each core handles 512 queries
against its batch's full 1024-point cloud.

Per-core pipeline (4 query-tiles of 128):
  1. PE: rotate all points into each query's frame (fp32 matmuls, K=3).
  2. ACT: scaled squares; GPSIMD: combine -> g = max((y^2+z^2)/R^2, x'^2/h^2).
  3. ACT: sign(1-g) (+count via accum); DVE: score = desc * sign;
     4x (max8 + match_replace) extracts the first 32 in-cylinder point indices.
  4. idx staging roundtrip through DRAM into the 16-partition-wrapped layout.
  5. SWDGE dma_gather (SBUF->SBUF, transpose): pulls Z-rows (256ch bf16 +
     xyz) for all 4096 samples, channel-major.
  6. PE: per-query A = R @ w1x^T/RAD (K=3); block-diag xyz rhs built via DRAM
     diagonal-stride staging; y1 = relu(Zg + A.x + b1) with identity-inject.
  7. PE: layer 2 (bf16); DVE: max over 32 samples; ACT: relu + bias; DMA out.
"""
import sys

_RL = "/opt/trn_rl_repo"
if _RL not in sys.path:
    sys.path.insert(0, _RL)

from contextlib import ExitStack

import numpy as np
import ml_dtypes

import concourse.bass as bass
import concourse.bacc as bacc
import concourse.mybir as mybir
import concourse.tile as tile
from concourse import library_config

F32 = mybir.dt.float32
BF16 = mybir.dt.bfloat16
F16 = mybir.dt.float16
I16 = mybir.dt.int16
U8 = mybir.dt.uint8
AL = mybir.AluOpType
AF = mybir.ActivationFunctionType
AX = mybir.AxisListType

RADIUS = 0.05
HMIN = -0.02
HMAX = 0.04
NS = 32
CIN = 512
CMID = 256
COUT = 256
EPS = 1e-5
B = 4
NQ = 1024
M = 1024
NT = 4          # query tiles per core
QPC = 512       # queries per core
G8 = 8          # groups per query tile
QG = 16         # queries per group
CX = (HMIN + HMAX) / 2.0
HH = (HMAX - HMIN) / 2.0
NCORES = 8

bf16 = ml_dtypes.bfloat16


def build_nc(_stage=4, reps=1, f32r=False, no_gather=False):
    nc = bacc.Bacc("TRN2", target_bir_lowering=False, num_devices=NCORES)
    I = {}

    def din(name, shape, dt):
        I[name] = nc.dram_tensor(name, shape, dt, kind="ExternalInput").ap()

    FR = mybir.dt.float32r
    din("xyzT", [3, M], F32)
    din("xyzP", [128, 8, 3], F32)
    din("feat", [CIN, M], FR)
    din("w1fT", [128, 4, CMID], FR)
    din("w1xA", [3, CMID], FR)
    din("RrowT", [3, NT, 3, 128], F32)
    din("RcolT", [3, NT, G8, 6 * QG], FR)
    din("Rq9", [128, NT, 9], F32)
    din("cQ9", [128, NT, 9], F32)
    din("cneg32", [3, QPC, NS], BF16)
    din("w2T", [128, 2, 2, 128], BF16)
    din("b1", [128, 2], F32)
    din("b2", [128, 2], F32)
    din("desc16", [128, M], F16)
    din("siota", [128, NS], F32)
    din("qglob", [128, NT, NS], I16)
    din("ident", [128, 128], BF16)
    OUT = nc.dram_tensor("out", [COUT, QPC], F32, kind="ExternalOutput").ap()

    with ExitStack() as ctx:
        tc = ctx.enter_context(tile.TileContext(nc))
        res = ctx.enter_context(tc.tile_pool(name="res", bufs=1))
        wk = ctx.enter_context(tc.tile_pool(name="wk", bufs=2))
        wk1 = ctx.enter_context(tc.tile_pool(name="wk1", bufs=1))
        drm = ctx.enter_context(tc.tile_pool(name="drm", bufs=2, space="DRAM"))
        drs = ctx.enter_context(tc.tile_pool(name="drs", bufs=1, space="DRAM"))
        p_rot = ctx.enter_context(tc.tile_pool(name="prot", bufs=1, space="PSUM"))
        p_y1 = ctx.enter_context(tc.tile_pool(name="py1", bufs=1, space="PSUM"))
        p_y2 = ctx.enter_context(tc.tile_pool(name="py2", bufs=2, space="PSUM"))
        p_sm = ctx.enter_context(tc.tile_pool(name="psml", bufs=1, space="PSUM"))

        nc.gpsimd.load_library(library_config.mlp)

        # ---------- residents ----------
        def rload(name, shape, dt, src=None):
            t = res.tile(shape, dt, tag="res_" + name)
            nc.sync.dma_start(out=t[:], in_=src if src is not None else I[name])
            return t

        xyzT_s = rload("xyzT", [3, M], F32)
        w1fT_s = rload("w1fT", [128, 4, CMID], FR)
        w1xA_s = rload("w1xA", [3, CMID], FR)
        RrowT_s = rload("RrowT", [3, NT, 3, 128], F32)
        RcolT_s = rload("RcolT", [3, NT, G8, 6 * QG], FR)
        Rq9_s = rload("Rq9", [128, NT, 9], F32)
        cQ9_s = rload("cQ9", [128, NT, 9], F32)
        w2_s = rload("w2T", [128, 2, 2, 128], BF16)
        b1_s = rload("b1", [128, 2], F32)
        b2_s = rload("b2", [128, 2], F32)
        desc_s = rload("desc16", [128, M], F16)
        siota_s = rload("siota", [128, NS], F32)
        qglob_s = rload("qglob", [128, NT, NS], I16)
        ident_s = rload("ident", [128, 128], BF16)

        feat_s = res.tile([128, 4, M], FR, tag='res_feat')
        for kc in range(4):
            nc.sync.dma_start(out=feat_s[:, kc], in_=I["feat"][kc * 128:(kc + 1) * 128, :])

        # ---------- Z table (point-major, bf16): [m%128, m//128, 384] ----------
        repctx = tc.For_i(0, reps, 1) if reps > 1 else None
        if repctx is not None:
            repctx.__enter__()
        ZT = res.tile([128, 8, 384], BF16, tag='res_ZT')
        nc.scalar.memzero(ZT[:])
        for mt in range(8):
            pz = p_sm.tile([128, CMID], F32, tag="sm")
            for kc in range(4):
                nc.tensor.matmul(out=pz[:], lhsT=feat_s[:, kc, mt * 128:(mt + 1) * 128],
                                 rhs=w1fT_s[:, kc], start=(kc == 0), stop=(kc == 3))
            nc.scalar.copy(out=ZT[:, mt, 0:CMID], in_=pz[:])
        nc.gpsimd.dma_start(out=ZT[:, :, CMID:CMID + 3], in_=I["xyzP"])

        # ---------- block-diag DRAM staging (zeros written once) ----------
        zsrc = res.tile([96, G8 * 512], BF16, tag='res_zsrc')
        nc.vector.memset(zsrc[:], 0.0)
        rbdD = []
        for i in range(2):
            d = drs.tile([96, G8, 512], BF16, tag="rbdD%d" % i)
            nc.sync.dma_start(out=d[:], in_=zsrc[:])
            rbdD.append(d)

        # ---------- per query tile ----------
        for t in range(NT):
            # Rc[k] = sum_j R[k,j] * c[j]
            rc9 = wk.tile([128, 3, 3], F32, tag="rc9")
            nc.vector.tensor_tensor(
                out=rc9[:],
                in0=Rq9_s[:, t].rearrange("p (a b) -> p a b", a=3),
                in1=cQ9_s[:, t].rearrange("p (a b) -> p a b", a=3),
                op=AL.mult)
            rcb = wk.tile([128, 8], F32, tag="rcb")
            nc.vector.tensor_reduce(out=rcb[:, 0:3], in_=rc9[:], axis=AX.X, op=AL.add)
            nc.vector.tensor_scalar(out=rcb[:, 3:4], in0=rcb[:, 0:1],
                                    scalar1=-1.0 / HH, scalar2=-CX / HH,
                                    op0=AL.mult, op1=AL.add)
            nc.vector.tensor_scalar(out=rcb[:, 4:6], in0=rcb[:, 1:3],
                                    scalar1=-1.0 / RADIUS, scalar2=None, op0=AL.mult)

            # rotations + scaled squares
            sq = wk1.tile([128, 3, M], F32, tag="sq")
            for r in range(3):
                pr = p_rot.tile([128, M], F32, tag="rot")
                for h in range(2):
                    nc.tensor.matmul(out=pr[:, h * 512:(h + 1) * 512],
                                     lhsT=RrowT_s[:, t, r],
                                     rhs=xyzT_s[:, h * 512:(h + 1) * 512],
                                     start=True, stop=True)
                scale = (1.0 / HH) if r == 0 else (1.0 / RADIUS)
                nc.scalar.activation(out=sq[:, r], in_=pr[:], func=AF.Square,
                                     bias=rcb[:, 3 + r:4 + r], scale=scale)

            # g = max(y2+z2, x2); sign; score; count
            tgg = wk1.tile([128, 2, M], F32, tag="tg")
            nc.vector.tensor_tensor(out=tgg[:, 0], in0=sq[:, 1], in1=sq[:, 2], op=AL.add)
            nc.vector.tensor_tensor(out=tgg[:, 1], in0=tgg[:, 0], in1=sq[:, 0], op=AL.max)
            sgn = wk.tile([128, M], F16, tag="sgn")
            cnt_acc = wk.tile([128, 1], F32, tag="cnta")
            nc.scalar.activation(out=sgn[:], in_=tgg[:, 1], func=AF.Sign,
                                 bias=1.0, scale=-1.0, accum_out=cnt_acc[:])
            score = wk.tile([128, M], F16, tag="score")
            nc.vector.tensor_tensor(out=score[:], in0=desc_s[:], in1=sgn[:], op=AL.mult)

            # extract first-32 indices
            v32 = wk.tile([128, NS], F16, tag="v32")
            for r4 in range(4):
                nc.vector.max(out=v32[:, r4 * 8:(r4 + 1) * 8], in_=score[:])
                if r4 < 3:
                    nc.vector.match_replace(out=score[:],
                                            in_to_replace=v32[:, r4 * 8:(r4 + 1) * 8],
                                            in_values=score[:], imm_value=-3000.0)
            nc.vector.tensor_scalar(out=rcb[:, 6:7], in0=cnt_acc[:],
                                    scalar1=0.5, scalar2=512.0, op0=AL.mult, op1=AL.add)
            cond = wk.tile([128, NS], U8, tag="cond")
            nc.vector.tensor_scalar(out=cond[:], in0=siota_s[:], scalar1=rcb[:, 6:7],
                                    scalar2=None, op0=AL.is_lt)
            mvf = wk.tile([128, NS], F32, tag="mvf")
            nc.vector.tensor_scalar(out=mvf[:], in0=v32[:], scalar1=-1.0, scalar2=2048.0,
                                    op0=AL.mult, op1=AL.add)
            mvi = wk.tile([128, NS], I16, tag="mvi")
            nc.vector.tensor_scalar(out=mvi[:], in0=mvf[:], scalar1=1023.0, scalar2=None,
                                    op0=AL.min)
            idx16 = wk.tile([128, NS], I16, tag="idx16")
            nc.vector.tensor_copy(out=idx16[:], in_=qglob_s[:, t])
            nc.vector.copy_predicated(out=idx16[:], mask=cond[:], data=mvi[:])

            if _stage <= 1:
                dbg = wk.tile([128, NS], F32, tag="dbg")
                nc.vector.tensor_copy(out=dbg[:], in_=idx16[:])
                nc.sync.dma_start(out=OUT[0:128, t * NS:(t + 1) * NS], in_=dbg[:])
                continue
            # stage roundtrip -> wrapped idx layout, replicated to 8 core-blocks
            stage = drm.tile([128, NS], I16, tag="stage")
            nc.sync.dma_start(out=stage[:], in_=idx16[:])
            wrap = stage[:].rearrange("q (h p) -> p (q h)", h=2, p=16)
            idxw = wk.tile([128, 256], I16, tag="idxw")
            for blk in range(8):
                nc.sync.dma_start(out=idxw[blk * 16:(blk + 1) * 16, :], in_=wrap)

            # gather: channel-major [c%128, c//128, sample]; row 2 parts 0-2 = xyz
            if _stage == 15:
                dbg15 = wk.tile([128, 256], F32, tag="dbg15")
                nc.vector.tensor_copy(out=dbg15[:], in_=idxw[:])
                nc.sync.dma_start(out=OUT[0:128, t * 128:(t + 1) * 128], in_=dbg15[:, 0:128])
                continue
            gz = wk1.tile([128, 3, NT * M], BF16, tag="gz")
            if no_gather:
                # same byte volume moved by plain DMA: isolates gather-specific cost
                gzf = gz[:].rearrange("p r m -> p (r m)")
                ztf = ZT[:].rearrange("p a b -> p (a b)")
                for rep4 in range(4):
                    nc.sync.dma_start(out=gzf[:, rep4 * 3072:(rep4 + 1) * 3072],
                                      in_=ztf[:])
            else:
                nc.gpsimd.dma_gather(out_ap=gz[:], in_ap=ZT[:], idxs_ap=idxw[:],
                                     num_idxs=4096, num_idxs_reg=4096, elem_size=384,
                                     transpose=True, sbuf_tokens_per_rank=128,
                                     sbuf_free_dim_per_rank=768, single_packet=False)

            if _stage <= 2:
                dbg2 = wk.tile([128, 128], F32, tag="dbg2")
                nc.vector.tensor_copy(out=dbg2[:], in_=gz[:, 0, 0:128])
                nc.sync.dma_start(out=OUT[0:128, t * 128:(t + 1) * 128], in_=dbg2[:])
                continue
            # A matrices, [row=(6q'+jj), o]
            Ab = wk.tile([96, G8, CMID], BF16, tag="Ab")
            for g in range(G8):
                pa = p_sm.tile([128, CMID], F32, tag="sm")
                nc.tensor.matmul(out=pa[0:96, :], lhsT=RcolT_s[:, t, g],
                                 rhs=w1xA_s[:], start=True, stop=True)
                nc.scalar.copy(out=Ab[:, g], in_=pa[0:96, :])

            # block-diagonal rhs via DRAM diagonal strides
            rd = rbdD[t % 2]
            rflat = rd[:].rearrange("r g c -> (r g c)")
            xyzD = drm.tile([3, NT * M], BF16, tag="xyzD")
            nc.sync.dma_start(out=xyzD[:], in_=gz[0:3, 2, :])
            for jj in range(3):
                srcx = xyzD[jj, :].rearrange("(g q s) -> q g s", g=G8, q=QG, s=NS)
                dstx = bass.AP(tensor=rflat.tensor, offset=rflat.offset + jj * 4096,
                               ap=[[6 * 4096 + 32, 16], [512, 8], [1, 32]])
                nc.sync.dma_start(out=dstx, in_=srcx)
                srcc = I["cneg32"][jj, t * 128:(t + 1) * 128, :].rearrange(
                    "(g q) s -> q g s", g=G8, q=QG)
                dstc = bass.AP(tensor=rflat.tensor, offset=rflat.offset + (3 + jj) * 4096,
                               ap=[[6 * 4096 + 32, 16], [512, 8], [1, 32]])
                nc.sync.dma_start(out=dstc, in_=srcc)
            rbd = wk.tile([96, G8, 512], BF16, tag="rbd")
            nc.sync.dma_start(out=rbd[:], in_=rd[:])

            # y1 = relu(Zg + A.x + b1)
            y1 = wk1.tile([128, 2, NT * M], BF16, tag="y1")
            for oc in range(2):
                for g in range(G8):
                    py1 = p_y1.tile([128, 512], F32, tag="y1p")
                    nc.tensor.matmul(out=py1[:], lhsT=ident_s[:],
                                     rhs=gz[:, oc, g * 512:(g + 1) * 512],
                                     start=True, stop=False)
                    nc.tensor.matmul(out=py1[:], lhsT=Ab[:, g, oc * 128:(oc + 1) * 128],
                                     rhs=rbd[:, g], start=False, stop=True)
                    nc.scalar.activation(out=y1[:, oc, g * 512:(g + 1) * 512], in_=py1[:],
                                         func=AF.Relu, bias=b1_s[:, oc:oc + 1], scale=1.0)

            if _stage <= 3:
                dbg3 = wk.tile([128, 128], F32, tag="dbg3")
                nc.vector.tensor_copy(out=dbg3[:], in_=y1[:, 0, 0:128])
                nc.sync.dma_start(out=OUT[0:128, t * 128:(t + 1) * 128], in_=dbg3[:])
                continue
            # layer 2 + max over 32 samples + relu+bias
            mx = wk.tile([128, 2, 128], F32, tag="mx")
            for oc2 in range(2):
                for gp in range(G8 // 2):
                    py2 = p_y2.tile([128, 1024], F32, tag="y2p")
                    for kc in range(2):
                        for ns in range(2):
                            nc.tensor.matmul(
                                out=py2[:, ns * 512:(ns + 1) * 512],
                                lhsT=w2_s[:, kc, oc2],
                                rhs=y1[:, kc, (2 * gp + ns) * 512:(2 * gp + ns + 1) * 512],
                                start=(kc == 0), stop=(kc == 1))
                    nc.vector.tensor_reduce(
                        out=mx[:, oc2, gp * 2 * QG:(gp + 1) * 2 * QG],
                        in_=py2[:].rearrange("p (q s) -> p q s", s=NS),
                        axis=AX.X, op=AL.max)
            outv = wk.tile([128, 2, 128], F32, tag="outv")
            for oc2 in range(2):
                nc.scalar.activation(out=outv[:, oc2], in_=mx[:, oc2], func=AF.Relu,
                                     bias=b2_s[:, oc2:oc2 + 1], scale=1.0)
                nc.sync.dma_start(out=OUT[oc2 * 128:(oc2 + 1) * 128, t * 128:(t + 1) * 128],
                                  in_=outv[:, oc2])
        if repctx is not None:
            repctx.__exit__(None, None, None)
    return nc


def prep_inputs(inputs):
    xyz = np.asarray(inputs["seed_xyz"], dtype=np.float32)
    feat = np.asarray(inputs["seed_features"], dtype=np.float32)
    rot = np.asarray(inputs["vp_rot"], dtype=np.float32)

    def fold(g, b, m, v):
        s = (np.asarray(g, np.float32) /
             np.sqrt(np.asarray(v, np.float32) + np.float32(EPS))).astype(np.float32)
        return s, (np.asarray(b, np.float32) - np.asarray(m, np.float32) * s).astype(np.float32)

    s1, bb1 = fold(inputs["g1"], inputs["b1"], inputs["m1"], inputs["v1"])
    s2, bb2 = fold(inputs["g2"], inputs["b2"], inputs["m2"], inputs["v2"])
    w1s = (np.asarray(inputs["w1"], np.float32) * s1[:, None]).astype(np.float32)
    w1x, w1f = w1s[:, :3], w1s[:, 3:]
    w2s = (np.asarray(inputs["w2"], np.float32) * s2[:, None]).astype(np.float32)

    desc16 = np.broadcast_to(
        (2048.0 - np.arange(M, dtype=np.float32)).astype(np.float16), (128, M)).copy()
    siota = np.broadcast_to(np.arange(NS, dtype=np.float32), (128, NS)).copy()
    ident = np.eye(128, dtype=np.float32).astype(bf16)
    w2T = np.ascontiguousarray(
        w2s.T.reshape(2, 128, 2, 128).transpose(1, 0, 2, 3)).astype(bf16)
    w1fT = np.ascontiguousarray(w1f.T.reshape(4, 128, CMID).transpose(1, 0, 2))
    w1xA = np.ascontiguousarray(w1x.T / np.float32(RADIUS))
    b1h = np.ascontiguousarray(bb1.reshape(2, 128).T)
    b2h = np.ascontiguousarray(bb2.reshape(2, 128).T)

    ins = []
    for c in range(NCORES):
        b, half = c // 2, c % 2
        X = xyz[b]
        Rt = rot[b]
        qs = slice(half * QPC, (half + 1) * QPC)
        Rq = Rt[qs]          # (512, 3, 3)  R[j, k] (einsum 'bnsj,bnjk')
        cq = X[qs]           # (512, 3)
        d = {}
        d["xyzT"] = np.ascontiguousarray(X.T)
        d["xyzP"] = np.ascontiguousarray(X.reshape(8, 128, 3).transpose(1, 0, 2))
        d["feat"] = feat[b]
        d["w1fT"] = w1fT
        d["w1xA"] = w1xA
        # cylinder-query rotation: x_k = sum_j rot[k, j] rel_j
        Rr = Rq.reshape(NT, 128, 3, 3)   # [t, q, k(row), j(col)]
        d["RrowT"] = np.ascontiguousarray(Rr.transpose(3, 0, 2, 1))  # [j, t, k, q]
        # layer-1 rotation A[j, o] = sum_k rot[j, k] w1x[o, k] / RAD
        Rg = Rq.reshape(NT, G8, QG, 3, 3)  # [t, g, q', j, k]
        rcol = np.zeros((3, NT, G8, 6 * QG), np.float32)
        for jj in range(6):
            rcol[:, :, :, jj::6] = Rg[:, :, :, jj % 3, :].transpose(3, 0, 1, 2)
        d["RcolT"] = rcol
        d["Rq9"] = np.ascontiguousarray(Rq.reshape(NT, 128, 9).transpose(1, 0, 2))
        d["cQ9"] = np.ascontiguousarray(
            np.tile(cq.reshape(NT, 128, 3), (1, 1, 3)).transpose(1, 0, 2))
        d["cneg32"] = np.ascontiguousarray(
            np.broadcast_to((-cq.T)[:, :, None], (3, QPC, NS))).astype(bf16)
        d["w2T"] = w2T
        d["b1"] = b1h
        d["b2"] = b2h
        d["desc16"] = desc16
        d["siota"] = siota
        qg = (half * QPC + np.arange(QPC, dtype=np.int16)).reshape(NT, 128)
        d["qglob"] = np.ascontiguousarray(
            np.broadcast_to(qg.T[:, :, None], (128, NT, NS))).astype(np.int16)
        d["ident"] = ident
        ins.append(d)
    return ins


def assemble(results):
    out = np.zeros((B, COUT, NQ), np.float32)
    for c in range(NCORES):
        b, half = c // 2, c % 2
        out[b, :, half * QPC:(half + 1) * QPC] = results[c]["out"]
    return out


_COMPILED = None


def _get_compiled():
    global _COMPILED
    if _COMPILED is None:
        nc = build_nc()
        nc.compile()
        _COMPILED = nc
    return _COMPILED


def kernel(**inputs):
    """Full-input entry point: shards across 8 NeuronCores, returns (B, 256, N) f32."""
    from concourse.bass_utils import run_bass_kernel_spmd
    nc = _get_compiled()
    ins = prep_inputs(inputs)
    res = run_bass_kernel_spmd(nc, ins, core_ids=list(range(NCORES)))
    return assemble(res.results)



# revision 49
# speedup vs baseline: 3.3325x; 3.3325x over previous
"""Bass/Tile kernel for nn_CloudCrop (cylinder-query + gather + SharedMLP + max-pool).

Sharding: 8 cores = 4 batches x 2 query-halves. Each core handles 512 queries
against its batch's full 1024-point cloud.

Per-core pipeline (4 query-tiles of 128):
  1. PE: rotate all points into each query's frame (fp32 matmuls, K=3).
  2. ACT: scaled squares; GPSIMD: combine -> g = max((y^2+z^2)/R^2, x'^2/h^2).
  3. ACT: sign(1-g) (+count via accum); DVE: score = desc * sign;
     4x (max8 + match_replace) extracts the first 32 in-cylinder point indices.
  4. idx staging roundtrip through DRAM into the 16-partition-wrapped layout.
  5. SWDGE dma_gather (SBUF->SBUF, transpose): pulls Z-rows (256ch bf16 +
     xyz) for all 4096 samples, channel-major.
  6. PE: per-query A = R @ w1x^T/RAD (K=3); block-diag xyz rhs built via DRAM
     diagonal-stride staging; y1 = relu(Zg + A.x + b1) with identity-inject.
  7. PE: layer 2 (bf16); DVE: max over 32 samples; ACT: relu + bias; DMA out.
"""
import sys

_RL = "/opt/trn_rl_repo"
if _RL not in sys.path:
    sys.path.insert(0, _RL)

from contextlib import ExitStack

import numpy as np
import ml_dtypes

import concourse.bass as bass
import concourse.bacc as bacc
import concourse.mybir as mybir
import concourse.tile as tile
from concourse import library_config

F32 = mybir.dt.float32
BF16 = mybir.dt.bfloat16
F16 = mybir.dt.float16
I16 = mybir.dt.int16
U8 = mybir.dt.uint8
AL = mybir.AluOpType
AF = mybir.ActivationFunctionType
AX = mybir.AxisListType

RADIUS = 0.05
HMIN = -0.02
HMAX = 0.04
NS = 32
CIN = 512
CMID = 256
COUT = 256
EPS = 1e-5
B = 4
NQ = 1024
M = 1024
NT = 4          # query tiles per core
QPC = 512       # queries per core
G8 = 8          # groups per query tile
QG = 16         # queries per group
CX = (HMIN + HMAX) / 2.0
HH = (HMAX - HMIN) / 2.0
NCORES = 8

bf16 = ml_dtypes.bfloat16


def build_nc(_stage=4, reps=1, f32r=False, no_gather=False, gvar=0, db=True):
    nq = 4 if gvar in (1, 3) else (2 if gvar == 5 else 1)
    nc = bacc.Bacc("TRN2", target_bir_lowering=False, num_devices=NCORES,
                   num_swdge_queues=nq)
    I = {}

    def din(name, shape, dt):
        I[name] = nc.dram_tensor(name, shape, dt, kind="ExternalInput").ap()

    FR = mybir.dt.float32r
    din("xyzT", [3, M], F32)
    din("xyzP", [128, 8, 3], F32)
    din("feat", [CIN, M], FR)
    din("w1fT", [128, 4, CMID], FR)
    din("w1xA", [3, CMID], FR)
    din("RrowT", [3, NT, 3, 128], F32)
    din("RcolT", [3, NT, G8, 6 * QG], FR)
    din("Rq9", [128, NT, 9], F32)
    din("cQ9", [128, NT, 9], F32)
    din("cneg32", [3, QPC, NS], BF16)
    din("w2T", [128, 2, 2, 128], BF16)
    din("b1", [128, 2], F32)
    din("b2", [128, 2], F32)
    din("desc16", [128, M], F16)
    din("siota", [128, NS], F32)
    din("qglob", [128, NT, NS], I16)
    din("ident", [128, 128], BF16)
    OUT = nc.dram_tensor("out", [COUT, QPC], F32, kind="ExternalOutput").ap()

    with ExitStack() as ctx:
        tc = ctx.enter_context(tile.TileContext(nc))
        res = ctx.enter_context(tc.tile_pool(name="res", bufs=1))
        wk = ctx.enter_context(tc.tile_pool(name="wk", bufs=2))
        wk1 = ctx.enter_context(tc.tile_pool(name="wk1", bufs=1))
        drm = ctx.enter_context(tc.tile_pool(name="drm", bufs=2, space="DRAM"))
        drs = ctx.enter_context(tc.tile_pool(name="drs", bufs=1, space="DRAM"))
        p_rot = ctx.enter_context(tc.tile_pool(name="prot", bufs=1, space="PSUM"))
        p_y1 = ctx.enter_context(tc.tile_pool(name="py1", bufs=1, space="PSUM"))
        p_y2 = ctx.enter_context(tc.tile_pool(name="py2", bufs=2, space="PSUM"))
        p_sm = ctx.enter_context(tc.tile_pool(name="psml", bufs=1, space="PSUM"))

        nc.gpsimd.load_library(library_config.mlp)

        # ---------- residents ----------
        def rload(name, shape, dt, src=None):
            t = res.tile(shape, dt, tag="res_" + name)
            nc.sync.dma_start(out=t[:], in_=src if src is not None else I[name])
            return t

        xyzT_s = rload("xyzT", [3, M], F32)
        w1fT_s = rload("w1fT", [128, 4, CMID], FR)
        w1xA_s = rload("w1xA", [3, CMID], FR)
        RrowT_s = rload("RrowT", [3, NT, 3, 128], F32)
        RcolT_s = rload("RcolT", [3, NT, G8, 6 * QG], FR)
        Rq9_s = rload("Rq9", [128, NT, 9], F32)
        cQ9_s = rload("cQ9", [128, NT, 9], F32)
        w2_s = rload("w2T", [128, 2, 2, 128], BF16)
        b1_s = rload("b1", [128, 2], F32)
        b2_s = rload("b2", [128, 2], F32)
        desc_s = rload("desc16", [128, M], F16)
        siota_s = rload("siota", [128, NS], F32)
        qglob_s = rload("qglob", [128, NT, NS], I16)
        ident_s = rload("ident", [128, 128], BF16)

        feat_s = res.tile([128, 4, M], FR, tag='res_feat')
        for kc in range(4):
            nc.sync.dma_start(out=feat_s[:, kc], in_=I["feat"][kc * 128:(kc + 1) * 128, :])

        # ---------- Z table (point-major, bf16): [m%128, m//128, 384] ----------
        repctx = tc.For_i(0, reps, 1) if reps > 1 else None
        if repctx is not None:
            repctx.__enter__()
        ZT = res.tile([128, 8, 384], BF16, tag='res_ZT')
        nc.scalar.memzero(ZT[:])
        for mt in range(8):
            pz = p_sm.tile([128, CMID], F32, tag="sm")
            for kc in range(4):
                nc.tensor.matmul(out=pz[:], lhsT=feat_s[:, kc, mt * 128:(mt + 1) * 128],
                                 rhs=w1fT_s[:, kc], start=(kc == 0), stop=(kc == 3))
            nc.scalar.copy(out=ZT[:, mt, 0:CMID], in_=pz[:])
        nc.gpsimd.dma_start(out=ZT[:, :, CMID:CMID + 3], in_=I["xyzP"])

        # ---------- block-diag DRAM staging (zeros written once) ----------
        zsrc = res.tile([96, G8 * 512], BF16, tag='res_zsrc')
        nc.vector.memset(zsrc[:], 0.0)
        rbdD = []
        for i in range(2):
            d = drs.tile([96, G8, 512], BF16, tag="rbdD%d" % i)
            nc.sync.dma_start(out=d[:], in_=zsrc[:])
            rbdD.append(d)

        # ---------- per query tile ----------
        for t in range(NT):
            # Rc[k] = sum_j R[k,j] * c[j]
            rc9 = wk.tile([128, 3, 3], F32, tag="rc9")
            nc.vector.tensor_tensor(
                out=rc9[:],
                in0=Rq9_s[:, t].rearrange("p (a b) -> p a b", a=3),
                in1=cQ9_s[:, t].rearrange("p (a b) -> p a b", a=3),
                op=AL.mult)
            rcb = wk.tile([128, 8], F32, tag="rcb")
            nc.vector.tensor_reduce(out=rcb[:, 0:3], in_=rc9[:], axis=AX.X, op=AL.add)
            nc.vector.tensor_scalar(out=rcb[:, 3:4], in0=rcb[:, 0:1],
                                    scalar1=-1.0 / HH, scalar2=-CX / HH,
                                    op0=AL.mult, op1=AL.add)
            nc.vector.tensor_scalar(out=rcb[:, 4:6], in0=rcb[:, 1:3],
                                    scalar1=-1.0 / RADIUS, scalar2=None, op0=AL.mult)

            # rotations + scaled squares
            sq = wk1.tile([128, 3, M], F32, tag="sq")
            for r in range(3):
                pr = p_rot.tile([128, M], F32, tag="rot")
                for h in range(2):
                    nc.tensor.matmul(out=pr[:, h * 512:(h + 1) * 512],
                                     lhsT=RrowT_s[:, t, r],
                                     rhs=xyzT_s[:, h * 512:(h + 1) * 512],
                                     start=True, stop=True)
                scale = (1.0 / HH) if r == 0 else (1.0 / RADIUS)
                nc.scalar.activation(out=sq[:, r], in_=pr[:], func=AF.Square,
                                     bias=rcb[:, 3 + r:4 + r], scale=scale)

            # g = max(y2+z2, x2); sign; score; count
            tgg = wk1.tile([128, 2, M], F32, tag="tg")
            nc.vector.tensor_tensor(out=tgg[:, 0], in0=sq[:, 1], in1=sq[:, 2], op=AL.add)
            nc.vector.tensor_tensor(out=tgg[:, 1], in0=tgg[:, 0], in1=sq[:, 0], op=AL.max)
            sgn = wk.tile([128, M], F16, tag="sgn")
            cnt_acc = wk.tile([128, 1], F32, tag="cnta")
            nc.scalar.activation(out=sgn[:], in_=tgg[:, 1], func=AF.Sign,
                                 bias=1.0, scale=-1.0, accum_out=cnt_acc[:])
            score = wk.tile([128, M], F16, tag="score")
            nc.vector.tensor_tensor(out=score[:], in0=desc_s[:], in1=sgn[:], op=AL.mult)

            # extract first-32 indices
            v32 = wk.tile([128, NS], F16, tag="v32")
            for r4 in range(4):
                nc.vector.max(out=v32[:, r4 * 8:(r4 + 1) * 8], in_=score[:])
                if r4 < 3:
                    nc.vector.match_replace(out=score[:],
                                            in_to_replace=v32[:, r4 * 8:(r4 + 1) * 8],
                                            in_values=score[:], imm_value=-3000.0)
            nc.vector.tensor_scalar(out=rcb[:, 6:7], in0=cnt_acc[:],
                                    scalar1=0.5, scalar2=512.0, op0=AL.mult, op1=AL.add)
            cond = wk.tile([128, NS], U8, tag="cond")
            nc.vector.tensor_scalar(out=cond[:], in0=siota_s[:], scalar1=rcb[:, 6:7],
                                    scalar2=None, op0=AL.is_lt)
            mvf = wk.tile([128, NS], F32, tag="mvf")
            nc.vector.tensor_scalar(out=mvf[:], in0=v32[:], scalar1=-1.0, scalar2=2048.0,
                                    op0=AL.mult, op1=AL.add)
            mvi = wk.tile([128, NS], I16, tag="mvi")
            nc.vector.tensor_scalar(out=mvi[:], in0=mvf[:], scalar1=1023.0, scalar2=None,
                                    op0=AL.min)
            idx16 = wk.tile([128, NS], I16, tag="idx16")
            nc.vector.tensor_copy(out=idx16[:], in_=qglob_s[:, t])
            nc.vector.copy_predicated(out=idx16[:], mask=cond[:], data=mvi[:])

            if _stage <= 1:
                dbg = wk.tile([128, NS], F32, tag="dbg")
                nc.vector.tensor_copy(out=dbg[:], in_=idx16[:])
                nc.sync.dma_start(out=OUT[0:128, t * NS:(t + 1) * NS], in_=dbg[:])
                continue
            # stage roundtrip -> wrapped idx layout, replicated to 8 core-blocks
            stage = drm.tile([128, NS], I16, tag="stage")
            nc.sync.dma_start(out=stage[:], in_=idx16[:])
            wrap = stage[:].rearrange("q (h p) -> p (q h)", h=2, p=16)
            idxw = wk.tile([128, 256], I16, tag="idxw")
            for blk in range(8):
                nc.sync.dma_start(out=idxw[blk * 16:(blk + 1) * 16, :], in_=wrap)

            # gather: channel-major [c%128, c//128, sample]; row 2 parts 0-2 = xyz
            if _stage == 15:
                dbg15 = wk.tile([128, 256], F32, tag="dbg15")
                nc.vector.tensor_copy(out=dbg15[:], in_=idxw[:])
                nc.sync.dma_start(out=OUT[0:128, t * 128:(t + 1) * 128], in_=dbg15[:, 0:128])
                continue
            if gvar in (3, 4):
                gz4 = wk1.tile([128, 4, 3, 1024], BF16, tag="gz", bufs=2 if db else 1)
                for k in range(4):
                    nc.gpsimd.dma_gather(
                        out_ap=gz4[:, k], in_ap=ZT[:],
                        idxs_ap=idxw[:, 64 * k:64 * (k + 1)],
                        num_idxs=1024, num_idxs_reg=1024, elem_size=384,
                        transpose=True, sbuf_tokens_per_rank=128,
                        sbuf_free_dim_per_rank=768, single_packet=False,
                        queue_num=k if gvar == 3 else 0)

                def gzv(oc, g):
                    return gz4[:, g // 2, oc, (g % 2) * 512:(g % 2 + 1) * 512]
                gz_xyz = gz4[0:3, :, 2, :]
            elif gvar == 5:
                gz4 = wk1.tile([128, 2, 3, 2048], BF16, tag="gz", bufs=2 if db else 1)
                for k in range(2):
                    nc.gpsimd.dma_gather(
                        out_ap=gz4[:, k], in_ap=ZT[:],
                        idxs_ap=idxw[:, 128 * k:128 * (k + 1)],
                        num_idxs=2048, num_idxs_reg=2048, elem_size=384,
                        transpose=True, sbuf_tokens_per_rank=128,
                        sbuf_free_dim_per_rank=768, single_packet=False,
                        queue_num=k)

                def gzv(oc, g):
                    return gz4[:, g // 4, oc, (g % 4) * 512:(g % 4 + 1) * 512]
                gz_xyz = gz4[0:3, :, 2, :]
            else:
                gz = wk1.tile([128, 3, NT * M], BF16, tag="gz", bufs=2 if db else 1)

                def gzv(oc, g):
                    return gz[:, oc, g * 512:(g + 1) * 512]
                gz_xyz = gz[0:3, 2, :]
            if gvar == 3:
                pass
            elif no_gather:
                # same byte volume moved by plain DMA: isolates gather-specific cost
                gzf = gz[:].rearrange("p r m -> p (r m)")
                ztf = ZT[:].rearrange("p a b -> p (a b)")
                for rep4 in range(4):
                    nc.sync.dma_start(out=gzf[:, rep4 * 3072:(rep4 + 1) * 3072],
                                      in_=ztf[:])
            elif gvar == 0:
                nc.gpsimd.dma_gather(out_ap=gz[:], in_ap=ZT[:], idxs_ap=idxw[:],
                                     num_idxs=4096, num_idxs_reg=4096, elem_size=384,
                                     transpose=True, sbuf_tokens_per_rank=128,
                                     sbuf_free_dim_per_rank=768, single_packet=False)
            elif gvar == 1:
                # split: Z-gather elem 256 (skip 256B pad) + xyz gather elem 128
                nc.gpsimd.dma_gather(out_ap=gz[:, 0:2, :], in_ap=ZT[:], idxs_ap=idxw[:],
                                     num_idxs=4096, num_idxs_reg=4096, elem_size=256,
                                     transpose=True, sbuf_tokens_per_rank=128,
                                     sbuf_free_dim_per_rank=768,
                                     sbuf_free_dim_pad_per_rank=256,
                                     single_packet=False)
                nc.gpsimd.dma_gather(out_ap=gz[:, 2:3, :], in_ap=ZT[:], idxs_ap=idxw[:],
                                     num_idxs=4096, num_idxs_reg=4096, elem_size=128,
                                     transpose=True, sbuf_tokens_per_rank=128,
                                     sbuf_free_dim_per_rank=768,
                                     sbuf_free_dim_pad_per_rank=256,
                                     sbuf_byte_offset=512,
                                     single_packet=False, queue_num=1 % nq)
            elif gvar == 2:
                # bulk DMA for Z bytes + small xyz transpose-gather only
                gzf = gz[:, 0:2, :].rearrange("p r m -> p (r m)")
                ztf = ZT[:].rearrange("p a b -> p (a b)")
                for rep4 in range(4):
                    nc.sync.dma_start(out=gzf[:, rep4 * 2048:(rep4 + 1) * 2048],
                                      in_=ztf[:, 0:2048])
                nc.gpsimd.dma_gather(out_ap=gz[:, 2:3, :], in_ap=ZT[:], idxs_ap=idxw[:],
                                     num_idxs=4096, num_idxs_reg=4096, elem_size=128,
                                     transpose=True, sbuf_tokens_per_rank=128,
                                     sbuf_free_dim_per_rank=768,
                                     sbuf_free_dim_pad_per_rank=256,
                                     sbuf_byte_offset=512,
                                     single_packet=False)

            if _stage <= 2:
                dbg2 = wk.tile([128, 128], F32, tag="dbg2")
                nc.vector.tensor_copy(out=dbg2[:], in_=gzv(0, 0)[:, 0:128])
                nc.sync.dma_start(out=OUT[0:128, t * 128:(t + 1) * 128], in_=dbg2[:])
                continue
            # A matrices, [row=(6q'+jj), o]
            Ab = wk.tile([96, G8, CMID], BF16, tag="Ab")
            for g in range(G8):
                pa = p_sm.tile([128, CMID], F32, tag="sm")
                nc.tensor.matmul(out=pa[0:96, :], lhsT=RcolT_s[:, t, g],
                                 rhs=w1xA_s[:], start=True, stop=True)
                nc.scalar.copy(out=Ab[:, g], in_=pa[0:96, :])

            # block-diagonal rhs via DRAM diagonal strides
            rd = rbdD[t % 2]
            rflat = rd[:].rearrange("r g c -> (r g c)")
            xyzD = drm.tile([3, NT * M], BF16, tag="xyzD")
            nc.sync.dma_start(out=xyzD[:], in_=gz_xyz)
            for jj in range(3):
                srcx = xyzD[jj, :].rearrange("(g q s) -> q g s", g=G8, q=QG, s=NS)
                dstx = bass.AP(tensor=rflat.tensor, offset=rflat.offset + jj * 4096,
                               ap=[[6 * 4096 + 32, 16], [512, 8], [1, 32]])
                nc.sync.dma_start(out=dstx, in_=srcx)
                srcc = I["cneg32"][jj, t * 128:(t + 1) * 128, :].rearrange(
                    "(g q) s -> q g s", g=G8, q=QG)
                dstc = bass.AP(tensor=rflat.tensor, offset=rflat.offset + (3 + jj) * 4096,
                               ap=[[6 * 4096 + 32, 16], [512, 8], [1, 32]])
                nc.sync.dma_start(out=dstc, in_=srcc)
            rbd = wk.tile([96, G8, 512], BF16, tag="rbd")
            nc.sync.dma_start(out=rbd[:], in_=rd[:])

            # y1 = relu(Zg + A.x + b1)
            y1 = wk1.tile([128, 2, NT * M], BF16, tag="y1", bufs=2 if db else 1)
            for oc in range(2):
                for g in range(G8):
                    py1 = p_y1.tile([128, 512], F32, tag="y1p")
                    nc.tensor.matmul(out=py1[:], lhsT=ident_s[:],
                                     rhs=gzv(oc, g),
                                     start=True, stop=False)
                    nc.tensor.matmul(out=py1[:], lhsT=Ab[:, g, oc * 128:(oc + 1) * 128],
                                     rhs=rbd[:, g], start=False, stop=True)
                    nc.scalar.activation(out=y1[:, oc, g * 512:(g + 1) * 512], in_=py1[:],
                                         func=AF.Relu, bias=b1_s[:, oc:oc + 1], scale=1.0)

            if _stage <= 3:
                dbg3 = wk.tile([128, 128], F32, tag="dbg3")
                nc.vector.tensor_copy(out=dbg3[:], in_=y1[:, 0, 0:128])
                nc.sync.dma_start(out=OUT[0:128, t * 128:(t + 1) * 128], in_=dbg3[:])
                continue
            # layer 2 + max over 32 samples + relu+bias
            mx = wk.tile([128, 2, 128], F32, tag="mx")
            for oc2 in range(2):
                for gp in range(G8 // 2):
                    py2 = p_y2.tile([128, 1024], F32, tag="y2p")
                    for kc in range(2):
                        for ns in range(2):
                            nc.tensor.matmul(
                                out=py2[:, ns * 512:(ns + 1) * 512],
                                lhsT=w2_s[:, kc, oc2],
                                rhs=y1[:, kc, (2 * gp + ns) * 512:(2 * gp + ns + 1) * 512],
                                start=(kc == 0), stop=(kc == 1))
                    nc.vector.tensor_reduce(
                        out=mx[:, oc2, gp * 2 * QG:(gp + 1) * 2 * QG],
                        in_=py2[:].rearrange("p (q s) -> p q s", s=NS),
                        axis=AX.X, op=AL.max)
            outv = wk.tile([128, 2, 128], F32, tag="outv")
            for oc2 in range(2):
                nc.scalar.activation(out=outv[:, oc2], in_=mx[:, oc2], func=AF.Relu,
                                     bias=b2_s[:, oc2:oc2 + 1], scale=1.0)
                nc.sync.dma_start(out=OUT[oc2 * 128:(oc2 + 1) * 128, t * 128:(t + 1) * 128],
                                  in_=outv[:, oc2])
        if repctx is not None:
            repctx.__exit__(None, None, None)
    return nc


def build_v2(reps=1, f32r_rot=False, _stage=4, _resmask=7):
    """Restructured kernel: host-prepped A-matrices (64-row block-diag with
    static ones rows), resident rbd double-buffer, trimmed DRAM staging,
    4-queue split gather, f16 phase-A mask math, pipelined A/B emission."""
    nc = bacc.Bacc("TRN2", target_bir_lowering=False, num_devices=NCORES)
    I = {}

    def din(name, shape, dt):
        I[name] = nc.dram_tensor(name, shape, dt, kind="ExternalInput").ap()

    FR = mybir.dt.float32r
    ROTDT = FR if f32r_rot else F32
    din("xyzT", [3, M], F32)
    din("xyzP", [128, 8, 3], F32)
    din("feat", [CIN, M], FR)
    din("w1fT", [128, 4, CMID], FR)
    din("RrowT", [3, NT, 3, 128], ROTDT)
    din("bias3", [128, NT, 3], F32)
    din("Ab4", [64, NT, G8, CMID], BF16)
    din("onesD", [16, G8, 512], BF16)
    din("w2T", [128, 2, 2, 128], BF16)
    din("b1", [128, 2], F32)
    din("b2", [128, 2], F32)
    din("desc16", [128, M], F16)
    din("siota", [128, NS], F32)
    din("qglob", [128, NT, NS], I16)
    din("ident", [128, 128], BF16)
    OUT = nc.dram_tensor("out", [COUT, QPC], F32, kind="ExternalOutput").ap()

    with ExitStack() as ctx:
        tc = ctx.enter_context(tile.TileContext(nc))
        res = ctx.enter_context(tc.tile_pool(name="res", bufs=1))
        wk = ctx.enter_context(tc.tile_pool(name="wk", bufs=2))
        wka = ctx.enter_context(tc.tile_pool(name="wka", bufs=1))
        wkg = ctx.enter_context(tc.tile_pool(name="wkg", bufs=2))
        drm = ctx.enter_context(tc.tile_pool(name="drm", bufs=2, space="DRAM"))
        drs = ctx.enter_context(tc.tile_pool(name="drs", bufs=1, space="DRAM"))
        p_rot = ctx.enter_context(tc.tile_pool(name="prot", bufs=2, space="PSUM"))
        p_y1 = ctx.enter_context(tc.tile_pool(name="py1", bufs=2, space="PSUM"))
        p_y2 = ctx.enter_context(tc.tile_pool(name="py2", bufs=2, space="PSUM"))

        nc.gpsimd.load_library(library_config.mlp)

        def rload(name, shape, dt):
            t = res.tile(shape, dt, tag="res_" + name)
            nc.sync.dma_start(out=t[:], in_=I[name])
            return t

        xyzT_s = rload("xyzT", [3, M], F32)
        if _resmask & 8:
            w1fT_s = rload("w1fT", [128, 4, CMID], FR)
        if _resmask & 16:
            RrowT_s = rload("RrowT", [3, NT, 3, 128], ROTDT)
        if _resmask & 1:
            bias3_s = rload("bias3", [128, NT, 3], F32)
        if _resmask & 2:
            Ab4_s = rload("Ab4", [64, NT, G8, CMID], BF16)
        if _resmask & 4:
            onesD_s = rload("onesD", [16, G8, 512], BF16)
        w2_s = rload("w2T", [128, 2, 2, 128], BF16)
        b1_s = rload("b1", [128, 2], F32)
        b2_s = rload("b2", [128, 2], F32)
        desc_s = rload("desc16", [128, M], F16)
        siota_s = rload("siota", [128, NS], F32)
        qglob_s = rload("qglob", [128, NT, NS], I16)
        ident_s = rload("ident", [128, 128], BF16)

        feat_s = res.tile([128, 4, M], FR, tag="res_feat")
        if _resmask & 32:
            for kc in range(4):
                nc.sync.dma_start(out=feat_s[:, kc],
                                  in_=I["feat"][kc * 128:(kc + 1) * 128, :])

        # Z table (point-major): pad zeroed once; data rewritten per rep
        ZT = res.tile([128, 8, 384], BF16, tag="res_ZT")
        if _resmask & 64:
            nc.scalar.memzero(ZT[:])

        # rbd residents: rows 0:48 = per-tile xyz diagonal (from DRAM),
        # rows 48:64 = static ones diagonal
        rbd64 = []
        for i in range(2):
            t = res.tile([64, G8, 512], BF16, tag="res_rbd%d" % i)
            if _stage not in (10, 11):
                nc.sync.dma_start(out=t[48:64], in_=onesD_s[:])
            rbd64.append(t)

        # DRAM diagonal staging buffers (zeros written once)
        zsrc = res.tile([48, G8 * 512], BF16, tag="res_zsrc")
        rbdD = []
        if _stage not in (10, 11):
            nc.vector.memset(zsrc[:], 0.0)
        for i in range(2):
            d = drs.tile([48, G8, 512], BF16, tag="rbdD%d" % i)
            if _stage not in (10, 11):
                nc.sync.dma_start(out=d[:], in_=zsrc[:])
            rbdD.append(d)

        if _stage == 10:
            dbg10 = wk.tile([128, 128], F32, tag="dbg10")
            nc.vector.tensor_copy(out=dbg10[:], in_=desc_s[:, 0:128])
            nc.sync.dma_start(out=OUT[0:128, 128:256], in_=dbg10[:])

        repctx = tc.For_i(0, reps, 1) if reps > 1 else None
        if repctx is not None:
            repctx.__enter__()

        # ---------- ZT build ----------
        if _stage != 10:
            for mt in range(8):
                pz = p_y1.tile([128, 512], F32, tag="y1p")
                for kc in range(4):
                    nc.tensor.matmul(out=pz[:, 0:CMID],
                                     lhsT=feat_s[:, kc, mt * 128:(mt + 1) * 128],
                                     rhs=w1fT_s[:, kc], start=(kc == 0), stop=(kc == 3))
                nc.scalar.copy(out=ZT[:, mt, 0:CMID], in_=pz[:, 0:CMID])
            nc.gpsimd.dma_start(out=ZT[:, :, CMID:CMID + 3], in_=I["xyzP"])

        # ---------- phase A: selection for tile t ----------
        def phase_a(t):
            sq = wka.tile([128, 3, M], F32, tag="sq")
            for r in range(3):
                pr = p_rot.tile([128, M], F32, tag="rot")
                for h in range(2):
                    nc.tensor.matmul(out=pr[:, h * 512:(h + 1) * 512],
                                     lhsT=RrowT_s[:, t, r],
                                     rhs=xyzT_s[:, h * 512:(h + 1) * 512],
                                     start=True, stop=True)
                scale = (1.0 / HH) if r == 0 else (1.0 / RADIUS)
                nc.scalar.activation(out=sq[:, r], in_=pr[:], func=AF.Square,
                                     bias=bias3_s[:, t, r:r + 1], scale=scale)
            tgg = wka.tile([128, M], F32, tag="tg")
            nc.vector.tensor_tensor(out=tgg[:], in0=sq[:, 1], in1=sq[:, 2], op=AL.add)
            nc.vector.tensor_tensor(out=tgg[:], in0=tgg[:], in1=sq[:, 0], op=AL.max)
            sgn = wk.tile([128, M], F16, tag="sgn")
            cnt_acc = wk.tile([128, 1], F32, tag="cnta")
            nc.scalar.activation(out=sgn[:], in_=tgg[:], func=AF.Sign,
                                 bias=1.0, scale=-1.0, accum_out=cnt_acc[:])
            score = wk.tile([128, M], F16, tag="score")
            nc.vector.tensor_tensor(out=score[:], in0=desc_s[:], in1=sgn[:], op=AL.mult)
            if _stage == 11 and t == 0:
                cols = {"a": (score[:, 0:128], 0), "b": (desc_s[:, 0:128], 128),
                        "c": (sgn[:, 0:128], 256), "d": (sq[:, 0, 0:128], 384)}
                for nm, (src, off) in cols.items():
                    dbgx = wk.tile([128, 128], F32, tag="dbgx" + nm)
                    nc.vector.tensor_copy(out=dbgx[:], in_=src)
                    nc.sync.dma_start(out=OUT[0:128, off:off + 128], in_=dbgx[:])

            v32 = wk.tile([128, NS], F16, tag="v32")
            for r4 in range(4):
                nc.vector.max(out=v32[:, r4 * 8:(r4 + 1) * 8], in_=score[:])
                if r4 < 3:
                    nc.vector.match_replace(out=score[:],
                                            in_to_replace=v32[:, r4 * 8:(r4 + 1) * 8],
                                            in_values=score[:], imm_value=-3000.0)
            thr = wk.tile([128, 1], F32, tag="thr")
            nc.vector.tensor_scalar(out=thr[:], in0=cnt_acc[:],
                                    scalar1=0.5, scalar2=512.0, op0=AL.mult, op1=AL.add)
            cond = wk.tile([128, NS], U8, tag="cond")
            nc.vector.tensor_scalar(out=cond[:], in0=siota_s[:], scalar1=thr[:],
                                    scalar2=None, op0=AL.is_lt)
            mvf = wk.tile([128, NS], F32, tag="mvf")
            nc.vector.tensor_scalar(out=mvf[:], in0=v32[:], scalar1=-1.0, scalar2=2048.0,
                                    op0=AL.mult, op1=AL.add)
            mvi = wk.tile([128, NS], I16, tag="mvi")
            nc.vector.tensor_scalar(out=mvi[:], in0=mvf[:], scalar1=1023.0, scalar2=None,
                                    op0=AL.min)
            idx16 = wk.tile([128, NS], I16, tag="idx16")
            nc.vector.tensor_copy(out=idx16[:], in_=qglob_s[:, t])
            nc.vector.copy_predicated(out=idx16[:], mask=cond[:], data=mvi[:])

            # wrap roundtrip + replicate to 8 blocks
            stage = drm.tile([128, NS], I16, tag="stage")
            nc.sync.dma_start(out=stage[:], in_=idx16[:])
            wrap = stage[:].rearrange("q (h p) -> p (q h)", h=2, p=16)
            idxw = wk.tile([128, 256], I16, tag="idxw")
            for blk in range(8):
                nc.sync.dma_start(out=idxw[blk * 16:(blk + 1) * 16, :], in_=wrap)

            if _stage == 1:
                dbg = wk.tile([128, NS], F32, tag="dbg")
                nc.vector.tensor_copy(out=dbg[:], in_=idx16[:])
                nc.sync.dma_start(out=OUT[0:128, t * NS:(t + 1) * NS], in_=dbg[:])
            gz4 = wkg.tile([128, 3, NT * M], BF16, tag="gz")
            if _stage not in (10, 11):
                nc.gpsimd.dma_gather(out_ap=gz4[:], in_ap=ZT[:], idxs_ap=idxw[:],
                                     num_idxs=4096, num_idxs_reg=4096, elem_size=384,
                                     transpose=True, sbuf_tokens_per_rank=128,
                                     sbuf_free_dim_per_rank=768, single_packet=False)
            if _stage == 2:
                dbg2 = wk.tile([128, 128], F32, tag="dbg2")
                nc.vector.tensor_copy(out=dbg2[:], in_=gz4[:, 0, 0:128])
                nc.sync.dma_start(out=OUT[0:128, t * 128:(t + 1) * 128], in_=dbg2[:])
            return gz4

        # ---------- phase B: gather-consume + MLP for tile t ----------
        def phase_b(t, gz4):
            if _stage <= 2 or _stage in (10, 11):
                return
            rd = rbdD[t % 2]
            rb = rbd64[t % 2]
            xyzD = drm.tile([3, NT * M], BF16, tag="xyzD")
            nc.sync.dma_start(out=xyzD[:], in_=gz4[0:3, 2, :])
            rflat = rd[:].rearrange("r g c -> (r g c)")
            for jj in range(3):
                srcx = xyzD[jj, :].rearrange("(g q s) -> q g s", g=G8, q=QG, s=NS)
                dstx = bass.AP(tensor=rflat.tensor, offset=rflat.offset + jj * 4096,
                               ap=[[3 * 4096 + 32, 16], [512, 8], [1, 32]])
                nc.sync.dma_start(out=dstx, in_=srcx)
            nc.sync.dma_start(out=rb[0:48], in_=rd[:])
            if _stage == 25:
                if t == 0:
                    dbg25 = wk.tile([64, 512], F32, tag="dbg25")
                    nc.vector.tensor_copy(out=dbg25[:], in_=rb[:, 0, :])
                    nc.sync.dma_start(out=OUT[0:64, 0:512], in_=dbg25[:])
                return

            y1 = wkg.tile([128, 2, NT * M], BF16, tag="y1")
            for oc in range(2):
                for g in range(G8):
                    py1 = p_y1.tile([128, 512], F32, tag="y1p")
                    nc.tensor.matmul(out=py1[:], lhsT=ident_s[:],
                                     rhs=gz4[:, oc, g * 512:(g + 1) * 512],
                                     start=True, stop=False)
                    nc.tensor.matmul(out=py1[:], lhsT=Ab4_s[:, t, g, oc * 128:(oc + 1) * 128],
                                     rhs=rb[:, g], start=False, stop=True)
                    nc.scalar.activation(out=y1[:, oc, g * 512:(g + 1) * 512], in_=py1[:],
                                         func=AF.Relu, bias=b1_s[:, oc:oc + 1], scale=1.0)

            if _stage == 3:
                dbg3 = wk.tile([128, 128], F32, tag="dbg3")
                nc.vector.tensor_copy(out=dbg3[:], in_=y1[:, 0, 0:128])
                nc.sync.dma_start(out=OUT[0:128, t * 128:(t + 1) * 128], in_=dbg3[:])
                return
            mx = wk.tile([128, 2, 128], F32, tag="mx")
            for oc2 in range(2):
                for g in range(G8):
                    py2 = p_y2.tile([128, 512], F32, tag="y2p")
                    for kc in range(2):
                        nc.tensor.matmul(
                            out=py2[:],
                            lhsT=w2_s[:, kc, oc2],
                            rhs=y1[:, kc, g * 512:(g + 1) * 512],
                            start=(kc == 0), stop=(kc == 1))
                    nc.vector.tensor_reduce(
                        out=mx[:, oc2, g * QG:(g + 1) * QG],
                        in_=py2[:].rearrange("p (q s) -> p q s", s=NS),
                        axis=AX.X, op=AL.max)
            outv = wk.tile([128, 2, 128], F32, tag="outv")
            for oc2 in range(2):
                nc.scalar.activation(out=outv[:, oc2], in_=mx[:, oc2], func=AF.Relu,
                                     bias=b2_s[:, oc2:oc2 + 1], scale=1.0)
                nc.sync.dma_start(out=OUT[oc2 * 128:(oc2 + 1) * 128, t * 128:(t + 1) * 128],
                                  in_=outv[:, oc2])

        # software-pipelined emission: A0 A1 B0 A2 B1 A3 B2 B3
        if _stage != 10:
            gzs = {}
            gzs[0] = phase_a(0)
            gzs[1] = phase_a(1)
            phase_b(0, gzs.pop(0))
            gzs[2] = phase_a(2)
            phase_b(1, gzs.pop(1))
            gzs[3] = phase_a(3)
            phase_b(2, gzs.pop(2))
            phase_b(3, gzs.pop(3))

        if repctx is not None:
            repctx.__exit__(None, None, None)
    return nc


def prep_v2(inputs, ins_v1=None):
    """Per-core inputs for build_v2 (extends prep_inputs host prep)."""
    xyz = np.asarray(inputs["seed_xyz"], dtype=np.float32)
    rot = np.asarray(inputs["vp_rot"], dtype=np.float32)

    def fold(g, b, m, v):
        s = (np.asarray(g, np.float32) /
             np.sqrt(np.asarray(v, np.float32) + np.float32(EPS))).astype(np.float32)
        return s, (np.asarray(b, np.float32) - np.asarray(m, np.float32) * s).astype(np.float32)

    s1, _ = fold(inputs["g1"], inputs["b1"], inputs["m1"], inputs["v1"])
    w1s = (np.asarray(inputs["w1"], np.float32) * s1[:, None]).astype(np.float32)
    w1x = w1s[:, :3]

    base = prep_inputs(inputs) if ins_v1 is None else ins_v1
    onesD = np.zeros((16, G8, 512), np.float32)
    for qp in range(16):
        onesD[qp, :, qp * NS:(qp + 1) * NS] = 1.0
    onesD = onesD.astype(bf16)

    ins = []
    for c in range(NCORES):
        b, half = c // 2, c % 2
        qs = slice(half * QPC, (half + 1) * QPC)
        Rq = rot[b][qs]          # (512, 3, 3) [q, j, k]
        cq = xyz[b][qs]          # (512, 3)
        d = {k: v for k, v in base[c].items()
             if k in ("xyzT", "xyzP", "feat", "w1fT", "RrowT", "w2T", "b1", "b2",
                      "desc16", "siota", "qglob", "ident")}
        # bias3: -(Rc + CX)/HH for x row, -Rc/RAD for y,z rows
        Rc = np.einsum('qkj,qj->qk', Rq, cq)           # selection rotation R @ c
        bias3 = np.empty((QPC, 3), np.float32)
        bias3[:, 0] = -(Rc[:, 0] + CX) / HH
        bias3[:, 1:] = -Rc[:, 1:] / RADIUS
        d["bias3"] = np.ascontiguousarray(
            bias3.reshape(NT, 128, 3).transpose(1, 0, 2))
        # layer-1 rotation B[q, j, o] = sum_k R[q, j, k] w1x[o, k] / RAD
        B = np.einsum('qjk,ok->qjo', Rq, w1x) / np.float32(RADIUS)  # (512, 3, 256)
        ones_lhs = -np.einsum('qj,qjo->qo', cq, B)                  # (512, 256)
        Ab4 = np.zeros((64, NT, G8, CMID), np.float32)
        Bg = B.reshape(NT, G8, QG, 3, CMID)
        Og = ones_lhs.reshape(NT, G8, QG, CMID)
        for qp in range(QG):
            for j in range(3):
                Ab4[3 * qp + j] = Bg[:, :, qp, j]
            Ab4[48 + qp] = Og[:, :, qp]
        d["Ab4"] = Ab4.astype(bf16)
        d["onesD"] = onesD
        ins.append(d)
    return ins


def prep_inputs(inputs):
    xyz = np.asarray(inputs["seed_xyz"], dtype=np.float32)
    feat = np.asarray(inputs["seed_features"], dtype=np.float32)
    rot = np.asarray(inputs["vp_rot"], dtype=np.float32)

    def fold(g, b, m, v):
        s = (np.asarray(g, np.float32) /
             np.sqrt(np.asarray(v, np.float32) + np.float32(EPS))).astype(np.float32)
        return s, (np.asarray(b, np.float32) - np.asarray(m, np.float32) * s).astype(np.float32)

    s1, bb1 = fold(inputs["g1"], inputs["b1"], inputs["m1"], inputs["v1"])
    s2, bb2 = fold(inputs["g2"], inputs["b2"], inputs["m2"], inputs["v2"])
    w1s = (np.asarray(inputs["w1"], np.float32) * s1[:, None]).astype(np.float32)
    w1x, w1f = w1s[:, :3], w1s[:, 3:]
    w2s = (np.asarray(inputs["w2"], np.float32) * s2[:, None]).astype(np.float32)

    desc16 = np.broadcast_to(
        (2048.0 - np.arange(M, dtype=np.float32)).astype(np.float16), (128, M)).copy()
    siota = np.broadcast_to(np.arange(NS, dtype=np.float32), (128, NS)).copy()
    ident = np.eye(128, dtype=np.float32).astype(bf16)
    w2T = np.ascontiguousarray(
        w2s.T.reshape(2, 128, 2, 128).transpose(1, 0, 2, 3)).astype(bf16)
    w1fT = np.ascontiguousarray(w1f.T.reshape(4, 128, CMID).transpose(1, 0, 2))
    w1xA = np.ascontiguousarray(w1x.T / np.float32(RADIUS))
    b1h = np.ascontiguousarray(bb1.reshape(2, 128).T)
    b2h = np.ascontiguousarray(bb2.reshape(2, 128).T)

    ins = []
    for c in range(NCORES):
        b, half = c // 2, c % 2
        X = xyz[b]
        Rt = rot[b]
        qs = slice(half * QPC, (half + 1) * QPC)
        Rq = Rt[qs]          # (512, 3, 3)  R[j, k] (einsum 'bnsj,bnjk')
        cq = X[qs]           # (512, 3)
        d = {}
        d["xyzT"] = np.ascontiguousarray(X.T)
        d["xyzP"] = np.ascontiguousarray(X.reshape(8, 128, 3).transpose(1, 0, 2))
        d["feat"] = feat[b]
        d["w1fT"] = w1fT
        d["w1xA"] = w1xA
        # cylinder-query rotation: x_k = sum_j rot[k, j] rel_j
        Rr = Rq.reshape(NT, 128, 3, 3)   # [t, q, k(row), j(col)]
        d["RrowT"] = np.ascontiguousarray(Rr.transpose(3, 0, 2, 1))  # [j, t, k, q]
        # layer-1 rotation A[j, o] = sum_k rot[j, k] w1x[o, k] / RAD
        Rg = Rq.reshape(NT, G8, QG, 3, 3)  # [t, g, q', j, k]
        rcol = np.zeros((3, NT, G8, 6 * QG), np.float32)
        for jj in range(6):
            rcol[:, :, :, jj::6] = Rg[:, :, :, jj % 3, :].transpose(3, 0, 1, 2)
        d["RcolT"] = rcol
        d["Rq9"] = np.ascontiguousarray(Rq.reshape(NT, 128, 9).transpose(1, 0, 2))
        d["cQ9"] = np.ascontiguousarray(
            np.tile(cq.reshape(NT, 128, 3), (1, 1, 3)).transpose(1, 0, 2))
        d["cneg32"] = np.ascontiguousarray(
            np.broadcast_to((-cq.T)[:, :, None], (3, QPC, NS))).astype(bf16)
        d["w2T"] = w2T
        d["b1"] = b1h
        d["b2"] = b2h
        d["desc16"] = desc16
        d["siota"] = siota
        qg = (half * QPC + np.arange(QPC, dtype=np.int16)).reshape(NT, 128)
        d["qglob"] = np.ascontiguousarray(
            np.broadcast_to(qg.T[:, :, None], (128, NT, NS))).astype(np.int16)
        d["ident"] = ident
        ins.append(d)
    return ins


def assemble(results):
    out = np.zeros((B, COUT, NQ), np.float32)
    for c in range(NCORES):
        b, half = c // 2, c % 2
        out[b, :, half * QPC:(half + 1) * QPC] = results[c]["out"]
    return out


_COMPILED = None


def _get_compiled():
    global _COMPILED
    if _COMPILED is None:
        nc = build_nc()
        nc.compile()
        _COMPILED = nc
    return _COMPILED


def kernel(**inputs):
    """Full-input entry point: shards across 8 NeuronCores, returns (B, 256, N) f32."""
    from concourse.bass_utils import run_bass_kernel_spmd
    nc = _get_compiled()
    ins = prep_inputs(inputs)
    res = run_bass_kernel_spmd(nc, ins, core_ids=list(range(NCORES)))
    return assemble(res.results)

